# revision 33
# baseline (speedup 1.0000x reference)
"""Trainium2 Bass kernel for nn_CapRNNModelHelper (bi-GRU + capsule routing).

Sharding: data-parallel over batch across 8 cores (16 batch rows per core).
Everything else (embedding table, GRU weights, capsule weights) replicated.

Per-core pipeline (fp16 operands, f32 accumulation):
  1. indirect-DMA gather of fp16 embedding rows (token order s-major)
  2. PE-transpose (matmul vs identity) -> e.T  [300, ntok] fp16
  3. x_proj matmuls -> xprz + xpn (both fp16), biases folded, z negated
     so sigmoid gives w = 1-z directly
  4. chunk-parallel scan: PCH chunks per direction advance together in
     wide instructions; WU warmup steps rebuild each chunk's entry state
     from the previous chunk's tail (GRU forgets, so truncation error is
     tiny).  State h is fp16, written straight into the h buffer.
  5. capsule matmul (fp16) -> u_hat [sb, 160] fp16
  6. 5-iter dynamic routing; iteration 0 skips the c-multiply entirely
     (softmax is uniform and squash is scale-invariant)
  7. final linear -> out [16, 2]
"""

import numpy as np
from contextlib import ExitStack

import concourse.bass as bass
import concourse.tile as tile
from concourse import mybir
from concourse.bass import IndirectOffsetOnAxis
from concourse.bass_utils import run_bass_kernel_spmd
from concourse.tile_rust import add_dep_helper

F32 = mybir.dt.float32
F16 = mybir.dt.float16
F8 = mybir.dt.float8e4
I32 = mybir.dt.int32
AF = mybir.ActivationFunctionType
OP = mybir.AluOpType
AX = mybir.AxisListType

VOCAB, D_W, H, S, B = 50000, 300, 128, 256, 128
NUM_CAP, DIM_CAP, ROUTINGS, EPS = 10, 16, 5, 1e-7
NCORES = 8
BL = B // NCORES          # 16 batch rows per core
NTOK = S * BL             # 4096 tokens per core
NGRP = NTOK // 128        # 32 gather groups of 128 tokens
NCH = NTOK // 512         # 8 x_proj chunks of 512 tokens
KCH = [(0, 128), (128, 128), (256, 44)]   # D_W split
G3 = 3 * H                # 384

RZW = 4 * BL              # 64   per-step rz width [rf zf rb zb]
NW = 2 * BL               # 32   per-step n width [nf nb]
PCH = 16                  # parallel chunks per direction in the scan
CCH = S // PCH            # 16 steps per chunk
WU = 8                    # warmup steps (state rebuild) per chunk
EXT = S + 2 * WU          # padded xp timeline
PB = PCH * BL             # 256  scan state width per direction
NST = WU + CCH            # scan steps
# block index for (dir d, gate g): rz blocks 0..3, n blocks 0..1
_BLKRZ = {(0, 0): 0, (0, 1): 1, (1, 0): 2, (1, 1): 3}


def _sub(base, off, dims):
    """Manual AP: base is a [128, X] AP; append free dims after partition."""
    return bass.AP(tensor=base.tensor, offset=base.offset + off,
                   ap=[base.ap[0]] + dims)


def _v(t, dims, off=0):
    return bass.AP(tensor=t.tensor, offset=t.offset + off,
                   ap=[t.ap[0]] + dims)


def _split_waits(nc, cap=1):
    """Hoist excess sync waits onto standalone event-semaphore ops.

    The walrus build on this stack accepts only `cap` sync-wait commands
    per ISA instruction; Tile can attach several. Event-semaphore ops on
    the same engine execute in queue order, so hoisting preserves
    semantics.
    """
    n = 0
    for fn in nc.m.functions:
        for bb in fn.blocks:
            out = []
            for ins in bb.instructions:
                si = ins.sync_info
                if si is not None and len(si.on_wait) > cap:
                    waits = list(si.on_wait)
                    keep = waits[len(waits) - cap:] if cap else []
                    for w in waits[:len(waits) - cap] if cap else waits:
                        n += 1
                        out.append(mybir.InstEventSemaphore(
                            name=f"wsplit-{n}", engine=ins.engine,
                            ins=[], outs=[],
                            sync_info=mybir.SyncInfo(on_wait=[w],
                                                     on_update=[])))
                    ins.sync_info = mybir.SyncInfo(
                        on_wait=keep, on_update=list(si.on_update))
                out.append(ins)
            bb.instructions = out
    return n


def _build(zero_bhn: bool, debug: bool = False):
    nc = bass.Bass()
    xidx_d = nc.declare_dram_parameter("xidx", [128, NGRP], I32, False)
    emb_d = nc.declare_dram_parameter("emb", [VOCAB, D_W], F16, False)
    wih_d = nc.declare_dram_parameter("wih", [2, D_W, G3], F16, False)
    whh_d = nc.declare_dram_parameter("whh", [2, H, G3], F16, False)
    biasx_d = nc.declare_dram_parameter("biasx", [128, 6], F32, False)
    bhn_d = nc.declare_dram_parameter("bhn", [128, 2], F32, False)
    wcap_d = nc.declare_dram_parameter("wcap", [2, H, 160], F16, False)
    wlin_d = nc.declare_dram_parameter("wlin", [160, 2], F32, False)
    blin_d = nc.declare_dram_parameter("blin", [2, 1], F32, False)
    selB_d = nc.declare_dram_parameter("selB", [128, BL], F16, False)
    selT_d = nc.declare_dram_parameter("selT", [BL, 128], F16, False)
    ident_d = nc.declare_dram_parameter("ident", [128, 128], F32, False)
    out_d = nc.declare_dram_parameter("out", [BL, 2], F32, True)

    with tile.TileContext(nc) as tc, ExitStack() as ctx:
        const = ctx.enter_context(tc.tile_pool(name="const", bufs=1))
        bigxp = ctx.enter_context(tc.tile_pool(name="bigxp", bufs=1))
        bighs = ctx.enter_context(tc.tile_pool(name="bighs", bufs=1))
        work = ctx.enter_context(tc.tile_pool(name="work", bufs=3))

        # ---- constants to SBUF ----
        xidx = const.tile([128, NGRP], I32)
        nc.sync.dma_start(out=xidx[:], in_=xidx_d[:, :])
        whh = const.tile([128, 2, G3], F16)
        for d in range(2):
            nc.sync.dma_start(out=whh[:, d, :], in_=whh_d[d, :, :])
        biasx = const.tile([128, 6], F32)
        nc.sync.dma_start(out=biasx[:], in_=biasx_d[:, :])
        bhn = const.tile([128, 2], F32)
        nc.sync.dma_start(out=bhn[:], in_=bhn_d[:, :])
        wcap = const.tile([128, 2, 160], F16)
        for k in range(2):
            nc.sync.dma_start(out=wcap[:, k, :], in_=wcap_d[k, :, :])
        wlin = const.tile([128, 2, 2], F32)        # chunk0 [:128], chunk1 [:32]
        nc.sync.dma_start(out=wlin[:, 0, :], in_=wlin_d[0:128, :])
        nc.sync.dma_start(out=wlin[:32, 1, :], in_=wlin_d[128:160, :])
        blin = const.tile([2, 1], F32)
        nc.sync.dma_start(out=blin[:], in_=blin_d[:, :])
        selB = const.tile([128, BL], F16)
        nc.sync.dma_start(out=selB[:], in_=selB_d[:, :])
        selT = const.tile([BL, 128], F16)
        nc.sync.dma_start(out=selT[:], in_=selT_d[:, :])
        ident = const.tile([128, 128], F32)
        nc.sync.dma_start(out=ident[:], in_=ident_d[:, :])
        identb = const.tile([128, 128], F16)
        nc.scalar.copy(identb[:], ident[:])
        ident8 = const.tile([128, 128], F8)
        nc.scalar.copy(ident8[:], ident[:])
        epst = const.tile([128, 1], F32)
        nc.vector.memset(epst[:], EPS)

        xprz = bigxp.tile([128, EXT * RZW], F16)   # 35 KB/part
        xpn = bigxp.tile([128, EXT * NW], F16)     # 17.5 KB/part
        hbf = bighs.tile([128, 2 * S * BL], F16)   # 16 KB/part; f then b
        HB0 = S * BL
        # warmup pads force h -> 0 exactly: r=sigmoid(-30)=0, w=sigmoid(30)=1,
        # xn=0  =>  h' = h + w*(n - h) = n = tanh(0 + r*hn) = 0
        for p0 in (0, S + WU):
            for blk, val in ((0, -30.0), (1, 30.0), (2, -30.0), (3, 30.0)):
                nc.vector.memset(_sub(xprz[:], p0 * RZW + blk * BL,
                                      [[RZW, WU], [1, BL]]), val)
            nc.gpsimd.memset(_sub(xpn[:], p0 * NW, [[1, WU * NW]]), 0.0)

        # ---- phases B+C: gather + transpose + x_proj, 2 half passes ----
        HTOK = NTOK // 2
        with tc.tile_pool(name="bc", bufs=1) as bc, \
             tc.tile_pool(name="gat", bufs=16) as gat, \
             tc.tile_pool(name="ps_bc", bufs=1, space="PSUM") as ps_bc:
            wih = bc.tile([128, 2, 3, G3], F16)   # [kpart, dir, kchunk, gcol]
            for d in range(2):
                for k, (k0, kn) in enumerate(KCH):
                    nc.sync.dma_start(out=wih[:kn, d, k, :],
                                      in_=wih_d[d, k0:k0 + kn, :])
            for half in range(2):
                eT = [bc.tile([128, HTOK], F16, name=f"eT{k}", tag=f"eT{k}")
                      for k in range(3)]
                for i in range(NGRP // 2):
                    ig = half * (NGRP // 2) + i
                    g = gat.tile([128, D_W], F16, name="g", tag="g")
                    nc.gpsimd.indirect_dma_start(
                        out=g[:], out_offset=None,
                        in_=emb_d[:, :],
                        in_offset=IndirectOffsetOnAxis(ap=xidx[:, ig:ig + 1],
                                                       axis=0))
                    # chunks 0/1: hardware XBAR transpose on the (idle) DMA
                    # queues; chunk 2 (44 rows, not 128-multiple): PE path
                    for k, (k0, kn) in enumerate(KCH[:2]):
                        nc.sync.dma_start_transpose(
                            eT[k][:, i * 128:(i + 1) * 128],
                            g[:, k0:k0 + kn])
                    k0, kn = KCH[2]
                    pt = ps_bc.tile([128, 128], F32, tag="ptr", bufs=4)
                    nc.tensor.matmul(pt[:kn, :], lhsT=g[:, k0:k0 + kn],
                                     rhs=identb[:], start=True, stop=True)
                    if i % 2 == 0:
                        nc.vector.tensor_copy(
                            eT[2][:kn, i * 128:(i + 1) * 128], pt[:kn, :])
                    else:
                        nc.scalar.copy(
                            eT[2][:kn, i * 128:(i + 1) * 128], pt[:kn, :])
                for d in range(2):
                    for gt in range(3):
                        for ch in range(NCH // 2):
                            px = ps_bc.tile([128, 512], F32, tag="px", bufs=3)
                            for k, (k0, kn) in enumerate(KCH):
                                nc.tensor.matmul(
                                    px[:, :],
                                    lhsT=wih[:kn, d, k, gt * H:(gt + 1) * H],
                                    rhs=eT[k][:kn, ch * 512:(ch + 1) * 512],
                                    start=(k == 0), stop=(k == 2))
                            gch = half * (NCH // 2) + ch
                            src = _v(px, [[BL, 32], [1, BL]])
                            if gt < 2:
                                blk = _BLKRZ[(d, gt)]
                                dst = _sub(xprz[:], (WU + gch * 32) * RZW + blk * BL,
                                           [[RZW, 32], [1, BL]])
                                bcol = blk
                            else:
                                dst = _sub(xpn[:], (WU + gch * 32) * NW + d * BL,
                                           [[NW, 32], [1, BL]])
                                bcol = 4 + d
                            if (d * 3 + gt + ch) % 2 == 0:
                                nc.vector.tensor_scalar_add(
                                    dst, src, biasx[:, bcol:bcol + 1])
                            else:
                                nc.scalar.activation(
                                    dst, src, AF.Identity,
                                    bias=biasx[:, bcol:bcol + 1])

        # ---- phase D: chunked-parallel scan ----
        # PCH chunks per direction advance together in single wide
        # instructions; WU warmup slots rebuild each chunk's entry state from
        # the previous chunk's tail.  State is fp16 and written directly
        # into hbf once past warmup; during warmup it ping-pongs in hsc.
        hz = bighs.tile([128, 2 * PB], F16)
        nc.gpsimd.memset(hz[:], 0.0)
        hsc = [bighs.tile([128, 2 * PB], F16, name=f"hsc{i}") for i in range(2)]

        def h_off(d, k):
            """hbf column offset of direction d's state after step k>=WU."""
            j = k - WU
            return j * BL if d == 0 else HB0 + (CCH - 1 - j) * BL

        def h_fused(k):
            """Both directions' state after step k as one 3-dim AP."""
            if k < 0:
                return hz[:]
            if k < WU:
                return hsc[k % 2][:]
            o0 = h_off(0, k)
            return _sub(hbf[:], o0, [[h_off(1, k) - o0, 2],
                                     [CCH * BL, PCH], [1, BL]])

        def h_dir(d, k):
            """Direction d's state after step k (matmul rhs)."""
            if k < 0:
                return _sub(hz[:], d * PB, [[1, PB]])
            if k < WU:
                return _sub(hsc[k % 2][:], d * PB, [[1, PB]])
            return _sub(hbf[:], h_off(d, k), [[CCH * BL, PCH], [1, BL]])

        with tc.tile_pool(name="ps_scan", bufs=1, space="PSUM") as ps_sc:
            for k in range(NST):
                for d in range(2):
                    prz = ps_sc.tile([128, 2 * PB], F32, tag=f"prz{d}",
                                     bufs=2)
                    pn = ps_sc.tile([128, PB], F32, tag=f"pn{d}", bufs=2)
                    xo = k if d == 0 else (CCH - 1 + 2 * WU - k)
                    xr_ap = _sub(xprz[:],
                                 xo * RZW + (2 * BL if d == 1 else 0),
                                 [[BL, 2], [CCH * RZW, PCH], [1, BL]])
                    hprev = h_dir(d, k - 1)
                    mi = nc.tensor.matmul(prz[:], lhsT=identb[:], rhs=xr_ap,
                                          start=True, stop=False)
                    g_r = nc.tensor.matmul(prz[:, 0:PB],
                                           lhsT=whh[:, d, 0:H], rhs=hprev,
                                           start=False, stop=False)
                    add_dep_helper(g_r.ins, mi.ins, sync=False, reason="acc")
                    g_z = nc.tensor.matmul(prz[:, PB:2 * PB],
                                           lhsT=whh[:, d, H:2 * H], rhs=hprev,
                                           start=False, stop=True)
                    add_dep_helper(g_z.ins, g_r.ins, sync=False, reason="acc")
                    nc.tensor.matmul(pn[:], lhsT=whh[:, d, 2 * H:3 * H],
                                     rhs=hprev, start=True, stop=True)

                    rw = work.tile([128, 2 * PB], F16, tag=f"rw{d}")
                    nc.scalar.activation(rw[:], prz[:], AF.Sigmoid)
                    # off-critical-path: a = w*h, c = h - a  (gpsimd)
                    a_t = work.tile([128, PB], F16, tag=f"a{d}")
                    nc.gpsimd.tensor_tensor(a_t[:], rw[:, PB:2 * PB], hprev,
                                            op=OP.mult)
                    c_t = work.tile([128, PB], F16, tag=f"c{d}")
                    nc.gpsimd.tensor_tensor(c_t[:], hprev, a_t[:],
                                            op=OP.subtract)
                    # critical path: tn = pn*r, t2 = tn+xn, n = tanh(t2),
                    # b = w*n, h' = c + b
                    tn = work.tile([128, PB], F16, tag=f"tn{d}")
                    if zero_bhn:
                        nc.vector.tensor_tensor(tn[:], pn[:], rw[:, 0:PB],
                                                op=OP.mult)
                    else:
                        nc.vector.scalar_tensor_tensor(
                            tn[:], pn[:], bhn[:, d:d + 1], rw[:, 0:PB],
                            op0=OP.add, op1=OP.mult)
                    t2 = work.tile([128, PB], F16, tag=f"t2{d}")
                    xn_ap = _sub(xpn[:], xo * NW + d * BL,
                                 [[CCH * NW, PCH], [1, BL]])
                    nc.vector.tensor_add(_v(t2, [[BL, PCH], [1, BL]]),
                                         _v(tn, [[BL, PCH], [1, BL]]), xn_ap)
                    n_t = work.tile([128, PB], F16, tag=f"n{d}")
                    nc.scalar.activation(n_t[:], t2[:], AF.Tanh)
                    b_t = work.tile([128, PB], F16, tag=f"b{d}")
                    nc.vector.tensor_tensor(b_t[:], rw[:, PB:2 * PB], n_t[:],
                                            op=OP.mult)
                    heng = nc.vector if d == 0 else nc.gpsimd
                    heng.tensor_tensor(h_dir(d, k), c_t[:], b_t[:],
                                       op=OP.add)

        # ---- phases E/F/G: capsule + routing + linear ----
        with tc.tile_pool(name="ef", bufs=1) as ef, \
             tc.tile_pool(name="rp", bufs=1) as rp, \
             tc.tile_pool(name="ps_ef", bufs=1, space="PSUM") as ps_ef:
            # capsule u_hat [sb, 160] fp16, and iteration-0 sequence sum
            # (uniform softmax + scale-invariant squash => no c multiply)
            uh = ef.tile([128, NGRP * 160], F16)
            # 4 parallel selB-reduction chains, one PSUM bank each
            # (512-f32 bank stride keeps each chain's accumulation region
            # in its own bank)
            CH4 = [(0, 8), (8, 8), (16, 8), (24, 8)]
            pos = ps_ef.tile([BL, 4 * 512], F32, tag="pos", bufs=1)

            def chains(rhs_of):
                for q, (c0, cn) in enumerate(CH4):
                    for j in range(cn):
                        nc.tensor.matmul(pos[:, q * 512:q * 512 + 160],
                                         lhsT=selB[:], rhs=rhs_of(c0 + j),
                                         start=(j == 0), stop=(j == cn - 1))

            for c in range(NGRP):
                pu = ps_ef.tile([128, 160], F32, tag="pu", bufs=2)
                nc.tensor.matmul(pu[:], lhsT=hbf[:, c * 128:(c + 1) * 128],
                                 rhs=wcap[:, 0, :], start=True, stop=False)
                nc.tensor.matmul(pu[:], lhsT=_sub(hbf[:], HB0 + c * 128,
                                                  [[1, 128]]),
                                 rhs=wcap[:, 1, :], start=False, stop=True)
                if c % 2 == 0:
                    nc.vector.tensor_copy(uh[:, c * 160:(c + 1) * 160], pu[:])
                else:
                    nc.scalar.copy(uh[:, c * 160:(c + 1) * 160], pu[:])
            chains(lambda c: uh[:, c * 160:(c + 1) * 160])

            # routing state
            bl_t = rp.tile([128, NGRP * NUM_CAP], F32, tag="bl")
            outputs = rp.tile([BL, 160], F16, tag="outs")
            out32 = rp.tile([BL, 160], F32, tag="out32")
            tmp = rp.tile([128, NGRP * 160], F16, tag="tmp")
            po = rp.tile([BL, 160], F32, tag="po", bufs=2)

            for it in range(ROUTINGS):
                if it > 0:
                    # softmax over capsules (no max-sub: |bl| is small)
                    sb_t = rp.tile([128, NGRP * NUM_CAP], F32, tag="sb",
                                   bufs=2)
                    nc.scalar.activation(sb_t[:], bl_t[:], AF.Exp)
                    sm = rp.tile([128, NGRP], F32, tag="sm", bufs=2)
                    nc.vector.tensor_reduce(
                        sm[:], _v(sb_t, [[NUM_CAP, NGRP], [1, NUM_CAP]]),
                        axis=AX.X, op=OP.add)
                    rc = rp.tile([128, NGRP], F32, tag="rc", bufs=2)
                    nc.vector.reciprocal(rc[:], sm[:])
                    c_t = rp.tile([128, NGRP * NUM_CAP], F16, tag="c",
                                  bufs=2)
                    nc.vector.tensor_tensor(
                        _v(c_t, [[NUM_CAP, NGRP], [1, NUM_CAP]]),
                        _v(sb_t, [[NUM_CAP, NGRP], [1, NUM_CAP]]),
                        _v(rc, [[1, NGRP], [0, NUM_CAP]]), op=OP.mult)
                    # tmp = u_hat * c (c broadcast over dc), then reduce
                    # over s: within-group via 4 parallel matmul chains,
                    # across partitions via selB.  Slice q's chain starts
                    # as soon as its multiply lands.
                    for q, (c0, cn) in enumerate(CH4):
                        nc.vector.tensor_tensor(
                            _sub(tmp[:], c0 * 160,
                                 [[160, cn], [DIM_CAP, NUM_CAP],
                                  [1, DIM_CAP]]),
                            _sub(uh[:], c0 * 160,
                                 [[160, cn], [DIM_CAP, NUM_CAP],
                                  [1, DIM_CAP]]),
                            _sub(c_t[:], c0 * NUM_CAP,
                                 [[NUM_CAP, cn], [1, NUM_CAP], [0, DIM_CAP]]),
                            op=OP.mult)
                        for j in range(cn):
                            nc.tensor.matmul(
                                pos[:, q * 512:q * 512 + 160], lhsT=selB[:],
                                rhs=tmp[:, (c0 + j) * 160:(c0 + j + 1) * 160],
                                start=(j == 0), stop=(j == cn - 1))
                nc.vector.tensor_reduce(po[:], _v(pos, [[1, 160], [512, 4]]),
                                        axis=AX.X, op=OP.add)
                # squash via ln/exp (same ACT table as Exp -> no reloads):
                # 1/sqrt(s+eps) = exp(-0.5*ln(s+eps))
                sq = rp.tile([BL, 160], F32, tag="sq", bufs=2)
                nc.scalar.square(sq[:], po[:])
                ssum = rp.tile([BL, NUM_CAP], F32, tag="ssum", bufs=2)
                nc.vector.tensor_reduce(
                    ssum[:], _v(sq, [[DIM_CAP, NUM_CAP], [1, DIM_CAP]]),
                    axis=AX.X, op=OP.add)
                lns = rp.tile([BL, NUM_CAP], F32, tag="lns", bufs=2)
                nc.scalar.activation(lns[:], ssum[:], AF.Ln,
                                     bias=epst[:BL, 0:1])
                rs = rp.tile([BL, NUM_CAP], F32, tag="rs", bufs=2)
                nc.scalar.activation(rs[:], lns[:], AF.Exp, scale=-0.5)
                nc.vector.tensor_tensor(
                    _v(outputs, [[DIM_CAP, NUM_CAP], [1, DIM_CAP]]),
                    _v(po, [[DIM_CAP, NUM_CAP], [1, DIM_CAP]]),
                    _v(rs, [[1, NUM_CAP], [0, DIM_CAP]]), op=OP.mult)
                if it == ROUTINGS - 1:
                    nc.gpsimd.tensor_tensor(
                        _v(out32, [[DIM_CAP, NUM_CAP], [1, DIM_CAP]]),
                        _v(po, [[DIM_CAP, NUM_CAP], [1, DIM_CAP]]),
                        _v(rs, [[1, NUM_CAP], [0, DIM_CAP]]), op=OP.mult)

                if it < ROUTINGS - 1:
                    # broadcast outputs to all 128 partitions via selT matmul
                    pob = ps_ef.tile([128, 160], F32, tag="pob", bufs=1)
                    nc.tensor.matmul(pob[:], lhsT=selT[:], rhs=outputs[:],
                                     start=True, stop=True)
                    ob = rp.tile([128, 160], F16, tag="ob", bufs=2)
                    nc.scalar.copy(ob[:], pob[:])
                    # tmp = u_hat * ob (ob broadcast over chunks), then
                    # du = sum over dc; interleave halves so the first du
                    # reduce overlaps the second multiply
                    du = rp.tile([128, NGRP * NUM_CAP], F32, tag="du", bufs=2)
                    for g0, gn in ((0, 16), (16, 16)):
                        nc.vector.tensor_tensor(
                            _sub(tmp[:], g0 * 160, [[160, gn], [1, 160]]),
                            _sub(uh[:], g0 * 160, [[160, gn], [1, 160]]),
                            _v(ob, [[0, gn], [1, 160]]),
                            op=OP.mult)
                        nc.vector.tensor_reduce(
                            _sub(du[:], g0 * NUM_CAP,
                                 [[NUM_CAP, gn], [1, NUM_CAP]]),
                            _sub(tmp[:], g0 * 160,
                                 [[160, gn], [DIM_CAP, NUM_CAP],
                                  [1, DIM_CAP]]),
                            axis=AX.X, op=OP.add)
                    if it == 0:
                        nc.gpsimd.tensor_copy(bl_t[:], du[:])
                    else:
                        nc.gpsimd.tensor_tensor(bl_t[:], bl_t[:], du[:],
                                                op=OP.add)

            # final linear (one PSUM bank, three regions)
            pfin = ps_ef.tile([128, 3 * BL], F32, tag="pfin", bufs=1)
            nc.tensor.matmul(pfin[:, 0:BL], lhsT=out32[:, 0:128],
                             rhs=ident[:BL, :BL], start=True, stop=True)
            nc.tensor.matmul(pfin[:32, BL:2 * BL], lhsT=out32[:, 128:160],
                             rhs=ident[:BL, :BL], start=True, stop=True)
            capsT = rp.tile([128, 2 * BL], F32, tag="capsT")
            nc.vector.tensor_copy(capsT[:, 0:BL], pfin[:, 0:BL])
            nc.vector.tensor_copy(capsT[:32, BL:2 * BL], pfin[:32, BL:2 * BL])
            pf = pfin[0:2, 2 * BL:3 * BL]
            nc.tensor.matmul(pf, lhsT=wlin[:, 0, :], rhs=capsT[:, 0:BL],
                             start=True, stop=False)
            nc.tensor.matmul(pf, lhsT=wlin[:32, 1, :],
                             rhs=capsT[:32, BL:2 * BL],
                             start=False, stop=True)
            outT = rp.tile([2, BL], F32, tag="outT")
            nc.scalar.activation(outT[:], pf, AF.Identity,
                                 bias=blin[:, 0:1])
            dst = bass.AP(tensor=out_d, offset=0, ap=[[1, 2], [2, BL]])
            nc.sync.dma_start(out=dst, in_=outT[:])

    return nc


_CACHE = {}


def _get_nc(zero_bhn):
    if zero_bhn not in _CACHE:
        nc = _build(zero_bhn)
        _split_waits(nc)   # HW-path legalization (CoreSim path builds its own)
        _CACHE[zero_bhn] = nc
    return _CACHE[zero_bhn]


def _host_inputs(x, emb, w_ih_f, w_hh_f, b_ih_f, b_hh_f,
                 w_ih_b, w_hh_b, b_ih_b, b_hh_b, W_cap, W_lin, b_lin):
    """Build the per-core input maps (everything but xidx is shared)."""
    f32 = np.float32
    f16 = np.float16
    neg = np.ones((G3,), f32)
    neg[H:2 * H] = -1.0        # negate z gate (sigmoid -> 1-z)

    wih = np.stack([(w_ih_f.T * neg).astype(f16), (w_ih_b.T * neg).astype(f16)])
    whh = np.stack([(w_hh_f.T * neg).astype(f16), (w_hh_b.T * neg).astype(f16)])

    biasx = np.zeros((128, 6), f32)
    for d, (bi, bh) in enumerate([(b_ih_f, b_hh_f), (b_ih_b, b_hh_b)]):
        biasx[:, _BLKRZ[(d, 0)]] = (bi[0:H] + bh[0:H])
        biasx[:, _BLKRZ[(d, 1)]] = -(bi[H:2 * H] + bh[H:2 * H])
        biasx[:, 4 + d] = bi[2 * H:3 * H]
    bhn = np.zeros((128, 2), f32)
    bhn[:, 0] = b_hh_f[2 * H:3 * H]
    bhn[:, 1] = b_hh_b[2 * H:3 * H]
    zero_bhn = bool(np.all(bhn == 0.0))

    wcap = np.stack([W_cap[0:H, :].astype(f16), W_cap[H:2 * H, :].astype(f16)])
    selB = (np.arange(128)[:, None] % BL == np.arange(BL)[None, :]).astype(f16)
    selT = selB.T.copy().astype(f16)
    ident = np.eye(128, dtype=f32)

    shared = dict(emb=np.ascontiguousarray(emb).astype(f16), wih=wih, whh=whh,
                  biasx=biasx, bhn=bhn, wcap=wcap,
                  wlin=np.ascontiguousarray(W_lin, f32),
                  blin=np.ascontiguousarray(b_lin, f32).reshape(2, 1),
                  selB=selB, selT=selT, ident=ident)

    in_maps = []
    for c in range(NCORES):
        xl = np.asarray(x[c * BL:(c + 1) * BL, :])          # [BL, S]
        tok = xl.T.reshape(-1).astype(np.int32)             # s-major [NTOK]
        xidx = np.ascontiguousarray(tok.reshape(NGRP, 128).T)  # [128, NGRP]
        in_maps.append(dict(shared, xidx=xidx))
    return in_maps, zero_bhn


def kernel(**inputs):
    in_maps, zero_bhn = _host_inputs(**{k: np.asarray(v) for k, v in
                                        inputs.items()})
    nc = _get_nc(zero_bhn)
    res = run_bass_kernel_spmd(nc, in_maps, list(range(NCORES)))
    return np.concatenate([res.results[c]["out"] for c in range(NCORES)],
                          axis=0)


def _install_ntff_hook():
    """Shim the missing antenv.axon_hooks so trace=True works under axon."""
    import sys, types
    if "antenv.axon_hooks" in sys.modules:
        return
    mod = types.ModuleType("antenv.axon_hooks")
    _h = [None]
    mod.set_axon_ntff_profile_hook = lambda h: _h.__setitem__(0, h)
    mod.get_axon_ntff_profile_hook = lambda: _h[0]
    sys.modules["antenv.axon_hooks"] = mod
    import antenv
    antenv.axon_hooks = mod
    from trn_agent_boot.trn_boot import _ntff_profile_via_ctypes
    mod.set_axon_ntff_profile_hook(
        _ntff_profile_via_ctypes("/opt/axon/libaxon_pjrt.so"))


def kernel_profiled(**inputs):
    """Same as kernel() but with NTFF tracing; returns (out, result_obj)."""
    _install_ntff_hook()
    in_maps, zero_bhn = _host_inputs(**{k: np.asarray(v) for k, v in
                                        inputs.items()})
    nc = _get_nc(zero_bhn)
    res = run_bass_kernel_spmd(nc, in_maps, list(range(NCORES)), trace=True)
    out = np.concatenate([res.results[c]["out"] for c in range(NCORES)],
                         axis=0)
    return out, res


# revision 37
# speedup vs baseline: 1.1927x; 1.1927x over previous
"""Trainium2 Bass kernel for nn_CapRNNModelHelper (bi-GRU + capsule routing).

Sharding: data-parallel over batch across 8 cores (16 batch rows per core).
Everything else (embedding table, GRU weights, capsule weights) replicated.

Per-core pipeline (fp16 operands, f32 accumulation):
  1. indirect-DMA gather of fp16 embedding rows (token order s-major)
  2. PE-transpose (matmul vs identity) -> e.T  [300, ntok] fp16
  3. x_proj matmuls -> xprz + xpn (both fp16), biases folded, z negated
     so sigmoid gives w = 1-z directly
  4. chunk-parallel scan: PCH chunks per direction advance together in
     wide instructions; WU warmup steps rebuild each chunk's entry state
     from the previous chunk's tail (GRU forgets, so truncation error is
     tiny).  State h is fp16, written straight into the h buffer.
  5. capsule matmul (fp16) -> u_hat [sb, 160] fp16
  6. 5-iter dynamic routing; iteration 0 skips the c-multiply entirely
     (softmax is uniform and squash is scale-invariant)
  7. final linear -> out [16, 2]
"""

import numpy as np
from contextlib import ExitStack

import concourse.bass as bass
import concourse.tile as tile
from concourse import mybir
from concourse.bass import IndirectOffsetOnAxis
from concourse.bass_utils import run_bass_kernel_spmd
from concourse.tile_rust import add_dep_helper

F32 = mybir.dt.float32
F16 = mybir.dt.float16
F8 = mybir.dt.float8e4
I32 = mybir.dt.int32
AF = mybir.ActivationFunctionType
OP = mybir.AluOpType
AX = mybir.AxisListType

VOCAB, D_W, H, S, B = 50000, 300, 128, 256, 128
NUM_CAP, DIM_CAP, ROUTINGS, EPS = 10, 16, 5, 1e-7
NCORES = 8
BL = B // NCORES          # 16 batch rows per core
NTOK = S * BL             # 4096 tokens per core
NGRP = NTOK // 128        # 32 gather groups of 128 tokens
NCH = NTOK // 512         # 8 x_proj chunks of 512 tokens
KCH = [(0, 128), (128, 128), (256, 44)]   # D_W split
G3 = 3 * H                # 384

RZW = 4 * BL              # 64   per-step rz width [rf zf rb zb]
NW = 2 * BL               # 32   per-step n width [nf nb]
PCH = 16                  # parallel chunks per direction in the scan
CCH = S // PCH            # 16 steps per chunk
WU = 8                    # warmup steps (state rebuild) per chunk
EXT = S + 2 * WU          # padded xp timeline
PB = PCH * BL             # 256  scan state width per direction
NST = WU + CCH            # scan steps
# block index for (dir d, gate g): rz blocks 0..3, n blocks 0..1
_BLKRZ = {(0, 0): 0, (0, 1): 1, (1, 0): 2, (1, 1): 3}


def _sub(base, off, dims):
    """Manual AP: base is a [128, X] AP; append free dims after partition."""
    return bass.AP(tensor=base.tensor, offset=base.offset + off,
                   ap=[base.ap[0]] + dims)


def _v(t, dims, off=0):
    return bass.AP(tensor=t.tensor, offset=t.offset + off,
                   ap=[t.ap[0]] + dims)


def _split_waits(nc, cap=1):
    """Hoist excess sync waits onto standalone event-semaphore ops.

    The walrus build on this stack accepts only `cap` sync-wait commands
    per ISA instruction; Tile can attach several. Event-semaphore ops on
    the same engine execute in queue order, so hoisting preserves
    semantics.
    """
    n = 0
    for fn in nc.m.functions:
        for bb in fn.blocks:
            out = []
            for ins in bb.instructions:
                si = ins.sync_info
                if si is not None and len(si.on_wait) > cap:
                    waits = list(si.on_wait)
                    keep = waits[len(waits) - cap:] if cap else []
                    for w in waits[:len(waits) - cap] if cap else waits:
                        n += 1
                        out.append(mybir.InstEventSemaphore(
                            name=f"wsplit-{n}", engine=ins.engine,
                            ins=[], outs=[],
                            sync_info=mybir.SyncInfo(on_wait=[w],
                                                     on_update=[])))
                    ins.sync_info = mybir.SyncInfo(
                        on_wait=keep, on_update=list(si.on_update))
                out.append(ins)
            bb.instructions = out
    return n


def _build(zero_bhn: bool, debug: bool = False):
    nc = bass.Bass()
    xidx_d = nc.declare_dram_parameter("xidx", [128, NGRP], I32, False)
    emb_d = nc.declare_dram_parameter("emb", [VOCAB, D_W], F16, False)
    wih_d = nc.declare_dram_parameter("wih", [2, D_W, G3], F16, False)
    whh_d = nc.declare_dram_parameter("whh", [2, H, G3], F16, False)
    biasx_d = nc.declare_dram_parameter("biasx", [128, 6], F32, False)
    bhn_d = nc.declare_dram_parameter("bhn", [128, 2], F32, False)
    wcap_d = nc.declare_dram_parameter("wcap", [2, H, 160], F16, False)
    wlin_d = nc.declare_dram_parameter("wlin", [160, 2], F32, False)
    blin_d = nc.declare_dram_parameter("blin", [2, 1], F32, False)
    selB_d = nc.declare_dram_parameter("selB", [128, BL], F16, False)
    selT_d = nc.declare_dram_parameter("selT", [BL, 128], F16, False)
    ident_d = nc.declare_dram_parameter("ident", [128, 128], F32, False)
    out_d = nc.declare_dram_parameter("out", [BL, 2], F32, True)

    with tile.TileContext(nc) as tc, ExitStack() as ctx:
        const = ctx.enter_context(tc.tile_pool(name="const", bufs=1))
        bigxp = ctx.enter_context(tc.tile_pool(name="bigxp", bufs=1))
        bighs = ctx.enter_context(tc.tile_pool(name="bighs", bufs=1))
        work = ctx.enter_context(tc.tile_pool(name="work", bufs=3))

        # ---- constants to SBUF ----
        xidx = const.tile([128, NGRP], I32)
        nc.sync.dma_start(out=xidx[:], in_=xidx_d[:, :])
        whh = const.tile([128, 2, G3], F16)
        for d in range(2):
            nc.sync.dma_start(out=whh[:, d, :], in_=whh_d[d, :, :])
        biasx = const.tile([128, 6], F32)
        nc.sync.dma_start(out=biasx[:], in_=biasx_d[:, :])
        bhn = const.tile([128, 2], F32)
        nc.sync.dma_start(out=bhn[:], in_=bhn_d[:, :])
        wcap = const.tile([128, 2, 160], F16)
        for k in range(2):
            nc.sync.dma_start(out=wcap[:, k, :], in_=wcap_d[k, :, :])
        wlin = const.tile([128, 2, 2], F32)        # chunk0 [:128], chunk1 [:32]
        nc.sync.dma_start(out=wlin[:, 0, :], in_=wlin_d[0:128, :])
        nc.sync.dma_start(out=wlin[:32, 1, :], in_=wlin_d[128:160, :])
        blin = const.tile([2, 1], F32)
        nc.sync.dma_start(out=blin[:], in_=blin_d[:, :])
        selB = const.tile([128, BL], F16)
        nc.sync.dma_start(out=selB[:], in_=selB_d[:, :])
        selT = const.tile([BL, 128], F16)
        nc.sync.dma_start(out=selT[:], in_=selT_d[:, :])
        ident = const.tile([128, 128], F32)
        nc.sync.dma_start(out=ident[:], in_=ident_d[:, :])
        identb = const.tile([128, 128], F16)
        nc.scalar.copy(identb[:], ident[:])
        ident8 = const.tile([128, 128], F8)
        nc.scalar.copy(ident8[:], ident[:])
        epst = const.tile([128, 1], F32)
        nc.vector.memset(epst[:], EPS)

        xprz = bigxp.tile([128, EXT * RZW], F16)   # 35 KB/part
        xpn = bigxp.tile([128, EXT * NW], F16)     # 17.5 KB/part
        hbf = bighs.tile([128, 2 * S * BL], F16)   # 16 KB/part; f then b
        HB0 = S * BL
        # warmup pads force h -> 0 exactly: r=sigmoid(-30)=0, w=sigmoid(30)=1,
        # xn=0  =>  h' = h + w*(n - h) = n = tanh(0 + r*hn) = 0
        for p0 in (0, S + WU):
            for blk, val in ((0, -30.0), (1, 30.0), (2, -30.0), (3, 30.0)):
                nc.vector.memset(_sub(xprz[:], p0 * RZW + blk * BL,
                                      [[RZW, WU], [1, BL]]), val)
            nc.gpsimd.memset(_sub(xpn[:], p0 * NW, [[1, WU * NW]]), 0.0)

        # ---- phases B+C: gather + transpose + x_proj, 2 half passes ----
        HTOK = NTOK // 2
        with tc.tile_pool(name="bc", bufs=1) as bc, \
             tc.tile_pool(name="gat", bufs=16) as gat, \
             tc.tile_pool(name="ps_bc", bufs=1, space="PSUM") as ps_bc:
            wih = bc.tile([128, 2, 3, G3], F16)   # [kpart, dir, kchunk, gcol]
            for d in range(2):
                for k, (k0, kn) in enumerate(KCH):
                    nc.sync.dma_start(out=wih[:kn, d, k, :],
                                      in_=wih_d[d, k0:k0 + kn, :])
            for half in range(2):
                eT = [bc.tile([128, HTOK], F16, name=f"eT{k}", tag=f"eT{k}")
                      for k in range(3)]
                for i in range(NGRP // 2):
                    ig = half * (NGRP // 2) + i
                    g = gat.tile([128, D_W], F16, name="g", tag="g")
                    nc.gpsimd.indirect_dma_start(
                        out=g[:], out_offset=None,
                        in_=emb_d[:, :],
                        in_offset=IndirectOffsetOnAxis(ap=xidx[:, ig:ig + 1],
                                                       axis=0))
                    for k, (k0, kn) in enumerate(KCH):
                        pt = ps_bc.tile([128, 128], F32, tag="ptr", bufs=4)
                        nc.tensor.matmul(pt[:kn, :], lhsT=g[:, k0:k0 + kn],
                                         rhs=identb[:], start=True, stop=True)
                        if (i + k) % 2 == 0:
                            nc.vector.tensor_copy(
                                eT[k][:kn, i * 128:(i + 1) * 128], pt[:kn, :])
                        else:
                            nc.scalar.copy(
                                eT[k][:kn, i * 128:(i + 1) * 128], pt[:kn, :])
                for d in range(2):
                    for gt in range(3):
                        for ch in range(NCH // 2):
                            px = ps_bc.tile([128, 512], F32, tag="px", bufs=3)
                            for k, (k0, kn) in enumerate(KCH):
                                nc.tensor.matmul(
                                    px[:, :],
                                    lhsT=wih[:kn, d, k, gt * H:(gt + 1) * H],
                                    rhs=eT[k][:kn, ch * 512:(ch + 1) * 512],
                                    start=(k == 0), stop=(k == 2))
                            gch = half * (NCH // 2) + ch
                            src = _v(px, [[BL, 32], [1, BL]])
                            if gt < 2:
                                blk = _BLKRZ[(d, gt)]
                                dst = _sub(xprz[:], (WU + gch * 32) * RZW + blk * BL,
                                           [[RZW, 32], [1, BL]])
                                bcol = blk
                            else:
                                dst = _sub(xpn[:], (WU + gch * 32) * NW + d * BL,
                                           [[NW, 32], [1, BL]])
                                bcol = 4 + d
                            if (d * 3 + gt + ch) % 2 == 0:
                                nc.vector.tensor_scalar_add(
                                    dst, src, biasx[:, bcol:bcol + 1])
                            else:
                                nc.scalar.activation(
                                    dst, src, AF.Identity,
                                    bias=biasx[:, bcol:bcol + 1])

        # ---- phase D: chunked-parallel scan ----
        # PCH chunks per direction advance together in single wide
        # instructions; WU warmup slots rebuild each chunk's entry state from
        # the previous chunk's tail.  State is fp16 and written directly
        # into hbf once past warmup; during warmup it ping-pongs in hsc.
        hz = bighs.tile([128, 2 * PB], F16)
        nc.gpsimd.memset(hz[:], 0.0)
        hsc = [bighs.tile([128, 2 * PB], F16, name=f"hsc{i}") for i in range(2)]

        def h_off(d, k):
            """hbf column offset of direction d's state after step k>=WU."""
            j = k - WU
            return j * BL if d == 0 else HB0 + (CCH - 1 - j) * BL

        def h_fused(k):
            """Both directions' state after step k as one 3-dim AP."""
            if k < 0:
                return hz[:]
            if k < WU:
                return hsc[k % 2][:]
            o0 = h_off(0, k)
            return _sub(hbf[:], o0, [[h_off(1, k) - o0, 2],
                                     [CCH * BL, PCH], [1, BL]])

        def h_dir(d, k):
            """Direction d's state after step k (matmul rhs)."""
            if k < 0:
                return _sub(hz[:], d * PB, [[1, PB]])
            if k < WU:
                return _sub(hsc[k % 2][:], d * PB, [[1, PB]])
            return _sub(hbf[:], h_off(d, k), [[CCH * BL, PCH], [1, BL]])

        with tc.tile_pool(name="ps_scan", bufs=1, space="PSUM") as ps_sc:
            prev_b = prev_h = None
            for k in range(NST):
                for d in range(2):
                    prz = ps_sc.tile([128, 2 * PB], F32, tag=f"prz{d}",
                                     bufs=2)
                    pn = ps_sc.tile([128, PB], F32, tag=f"pn{d}", bufs=2)
                    xo = k if d == 0 else (CCH - 1 + 2 * WU - k)
                    xr_ap = _sub(xprz[:],
                                 xo * RZW + (2 * BL if d == 1 else 0),
                                 [[BL, 2], [CCH * RZW, PCH], [1, BL]])
                    hprev = h_dir(d, k - 1)
                    mi = nc.tensor.matmul(prz[:], lhsT=identb[:], rhs=xr_ap,
                                          start=True, stop=False)
                    g_r = nc.tensor.matmul(prz[:, 0:PB],
                                           lhsT=whh[:, d, 0:H], rhs=hprev,
                                           start=False, stop=False)
                    add_dep_helper(g_r.ins, mi.ins, sync=False, reason="acc")
                    g_z = nc.tensor.matmul(prz[:, PB:2 * PB],
                                           lhsT=whh[:, d, H:2 * H], rhs=hprev,
                                           start=False, stop=True)
                    add_dep_helper(g_z.ins, g_r.ins, sync=False, reason="acc")
                    nc.tensor.matmul(pn[:], lhsT=whh[:, d, 2 * H:3 * H],
                                     rhs=hprev, start=True, stop=True)

                    rw = work.tile([128, 2 * PB], F16, tag=f"rw{d}")
                    nc.scalar.activation(rw[:], prz[:], AF.Sigmoid)
                    # off-critical-path: a = w*h, c = h - a  (gpsimd)
                    a_t = work.tile([128, PB], F16, tag=f"a{d}")
                    nc.gpsimd.tensor_tensor(a_t[:], rw[:, PB:2 * PB], hprev,
                                            op=OP.mult)
                    c_t = work.tile([128, PB], F16, tag=f"c{d}")
                    nc.gpsimd.tensor_tensor(c_t[:], hprev, a_t[:],
                                            op=OP.subtract)
                    # critical path: tn = pn*r, t2 = tn+xn, n = tanh(t2),
                    # b = w*n, h' = c + b
                    tn = work.tile([128, PB], F16, tag=f"tn{d}")
                    if zero_bhn:
                        tni = nc.vector.tensor_tensor(tn[:], pn[:],
                                                      rw[:, 0:PB], op=OP.mult)
                    else:
                        tni = nc.vector.scalar_tensor_tensor(
                            tn[:], pn[:], bhn[:, d:d + 1], rw[:, 0:PB],
                            op0=OP.add, op1=OP.mult)
                    if d == 1 and prev_b is not None:
                        # keep d0's post-tanh ops ahead of d1's gate math in
                        # the in-order vector queue (shortens d0's h' path)
                        add_dep_helper(tni.ins, prev_b.ins, sync=False,
                                       reason="ord")
                    t2 = work.tile([128, PB], F16, tag=f"t2{d}")
                    xn_ap = _sub(xpn[:], xo * NW + d * BL,
                                 [[CCH * NW, PCH], [1, BL]])
                    t2i = nc.vector.tensor_add(_v(t2, [[BL, PCH], [1, BL]]),
                                               _v(tn, [[BL, PCH], [1, BL]]),
                                               xn_ap)
                    if d == 1 and prev_h is not None:
                        add_dep_helper(t2i.ins, prev_h.ins, sync=False,
                                       reason="ord")
                    n_t = work.tile([128, PB], F16, tag=f"n{d}")
                    nc.scalar.activation(n_t[:], t2[:], AF.Tanh)
                    b_t = work.tile([128, PB], F16, tag=f"b{d}")
                    prev_b = nc.vector.tensor_tensor(b_t[:], rw[:, PB:2 * PB],
                                                     n_t[:], op=OP.mult)
                    heng = nc.vector if d == 0 else nc.gpsimd
                    prev_h = heng.tensor_tensor(h_dir(d, k), c_t[:], b_t[:],
                                                op=OP.add)

        # ---- phases E/F/G: capsule + routing + linear ----
        with tc.tile_pool(name="ef", bufs=1) as ef, \
             tc.tile_pool(name="rp", bufs=1) as rp, \
             tc.tile_pool(name="ps_ef", bufs=1, space="PSUM") as ps_ef:
            # capsule u_hat [sb, 160] fp16, and iteration-0 sequence sum
            # (uniform softmax + scale-invariant squash => no c multiply)
            uh = ef.tile([128, NGRP * 160], F16)
            # 4 parallel selB-reduction chains, one PSUM bank each
            # (512-f32 bank stride keeps each chain's accumulation region
            # in its own bank)
            CH4 = [(0, 8), (8, 8), (16, 8), (24, 8)]
            pos = ps_ef.tile([BL, 4 * 512], F32, tag="pos", bufs=1)

            def chains(rhs_of):
                for q, (c0, cn) in enumerate(CH4):
                    for j in range(cn):
                        nc.tensor.matmul(pos[:, q * 512:q * 512 + 160],
                                         lhsT=selB[:], rhs=rhs_of(c0 + j),
                                         start=(j == 0), stop=(j == cn - 1))

            for c in range(NGRP):
                pu = ps_ef.tile([128, 160], F32, tag="pu", bufs=2)
                nc.tensor.matmul(pu[:], lhsT=hbf[:, c * 128:(c + 1) * 128],
                                 rhs=wcap[:, 0, :], start=True, stop=False)
                nc.tensor.matmul(pu[:], lhsT=_sub(hbf[:], HB0 + c * 128,
                                                  [[1, 128]]),
                                 rhs=wcap[:, 1, :], start=False, stop=True)
                if c % 2 == 0:
                    nc.vector.tensor_copy(uh[:, c * 160:(c + 1) * 160], pu[:])
                else:
                    nc.scalar.copy(uh[:, c * 160:(c + 1) * 160], pu[:])
            chains(lambda c: uh[:, c * 160:(c + 1) * 160])

            # routing state
            bl_t = rp.tile([128, NGRP * NUM_CAP], F32, tag="bl")
            outputs = rp.tile([BL, 160], F16, tag="outs")
            out32 = rp.tile([BL, 160], F32, tag="out32")
            tmp = rp.tile([128, NGRP * 160], F16, tag="tmp")
            po = rp.tile([BL, 160], F32, tag="po", bufs=2)

            for it in range(ROUTINGS):
                if it > 0:
                    # softmax over capsules (no max-sub: |bl| is small)
                    sb_t = rp.tile([128, NGRP * NUM_CAP], F32, tag="sb",
                                   bufs=2)
                    nc.scalar.activation(sb_t[:], bl_t[:], AF.Exp)
                    sm = rp.tile([128, NGRP], F32, tag="sm", bufs=2)
                    nc.vector.tensor_reduce(
                        sm[:], _v(sb_t, [[NUM_CAP, NGRP], [1, NUM_CAP]]),
                        axis=AX.X, op=OP.add)
                    rc = rp.tile([128, NGRP], F32, tag="rc", bufs=2)
                    nc.vector.reciprocal(rc[:], sm[:])
                    c_t = rp.tile([128, NGRP * NUM_CAP], F16, tag="c",
                                  bufs=2)
                    nc.vector.tensor_tensor(
                        _v(c_t, [[NUM_CAP, NGRP], [1, NUM_CAP]]),
                        _v(sb_t, [[NUM_CAP, NGRP], [1, NUM_CAP]]),
                        _v(rc, [[1, NGRP], [0, NUM_CAP]]), op=OP.mult)
                    # tmp = u_hat * c (c broadcast over dc), then reduce
                    # over s: within-group via 4 parallel matmul chains,
                    # across partitions via selB.  Slice q's chain starts
                    # as soon as its multiply lands; gpsimd takes the last
                    # slice concurrently with vector's three.
                    for q, (c0, cn) in enumerate(CH4):
                        eng = nc.gpsimd if q == 3 else nc.vector
                        eng.tensor_tensor(
                            _sub(tmp[:], c0 * 160,
                                 [[160, cn], [DIM_CAP, NUM_CAP],
                                  [1, DIM_CAP]]),
                            _sub(uh[:], c0 * 160,
                                 [[160, cn], [DIM_CAP, NUM_CAP],
                                  [1, DIM_CAP]]),
                            _sub(c_t[:], c0 * NUM_CAP,
                                 [[NUM_CAP, cn], [1, NUM_CAP], [0, DIM_CAP]]),
                            op=OP.mult)
                        for j in range(cn):
                            nc.tensor.matmul(
                                pos[:, q * 512:q * 512 + 160], lhsT=selB[:],
                                rhs=tmp[:, (c0 + j) * 160:(c0 + j + 1) * 160],
                                start=(j == 0), stop=(j == cn - 1))
                nc.vector.tensor_reduce(po[:], _v(pos, [[1, 160], [512, 4]]),
                                        axis=AX.X, op=OP.add)
                # squash via ln/exp (same ACT table as Exp -> no reloads):
                # 1/sqrt(s+eps) = exp(-0.5*ln(s+eps))
                sq = rp.tile([BL, 160], F32, tag="sq", bufs=2)
                nc.scalar.square(sq[:], po[:])
                ssum = rp.tile([BL, NUM_CAP], F32, tag="ssum", bufs=2)
                nc.vector.tensor_reduce(
                    ssum[:], _v(sq, [[DIM_CAP, NUM_CAP], [1, DIM_CAP]]),
                    axis=AX.X, op=OP.add)
                lns = rp.tile([BL, NUM_CAP], F32, tag="lns", bufs=2)
                nc.scalar.activation(lns[:], ssum[:], AF.Ln,
                                     bias=epst[:BL, 0:1])
                rs = rp.tile([BL, NUM_CAP], F32, tag="rs", bufs=2)
                nc.scalar.activation(rs[:], lns[:], AF.Exp, scale=-0.5)
                nc.vector.tensor_tensor(
                    _v(outputs, [[DIM_CAP, NUM_CAP], [1, DIM_CAP]]),
                    _v(po, [[DIM_CAP, NUM_CAP], [1, DIM_CAP]]),
                    _v(rs, [[1, NUM_CAP], [0, DIM_CAP]]), op=OP.mult)
                if it == ROUTINGS - 1:
                    nc.gpsimd.tensor_tensor(
                        _v(out32, [[DIM_CAP, NUM_CAP], [1, DIM_CAP]]),
                        _v(po, [[DIM_CAP, NUM_CAP], [1, DIM_CAP]]),
                        _v(rs, [[1, NUM_CAP], [0, DIM_CAP]]), op=OP.mult)

                if it < ROUTINGS - 1:
                    # broadcast outputs to all 128 partitions via selT matmul
                    pob = ps_ef.tile([128, 160], F32, tag="pob", bufs=1)
                    nc.tensor.matmul(pob[:], lhsT=selT[:], rhs=outputs[:],
                                     start=True, stop=True)
                    ob = rp.tile([128, 160], F16, tag="ob", bufs=2)
                    nc.scalar.copy(ob[:], pob[:])
                    # tmp = u_hat * ob (ob broadcast over chunks), then
                    # du = sum over dc; interleave halves so the first du
                    # reduce overlaps the second multiply
                    du = rp.tile([128, NGRP * NUM_CAP], F32, tag="du", bufs=2)
                    for g0, gn in ((0, 16), (16, 16)):
                        nc.vector.tensor_tensor(
                            _sub(tmp[:], g0 * 160, [[160, gn], [1, 160]]),
                            _sub(uh[:], g0 * 160, [[160, gn], [1, 160]]),
                            _v(ob, [[0, gn], [1, 160]]),
                            op=OP.mult)
                        nc.vector.tensor_reduce(
                            _sub(du[:], g0 * NUM_CAP,
                                 [[NUM_CAP, gn], [1, NUM_CAP]]),
                            _sub(tmp[:], g0 * 160,
                                 [[160, gn], [DIM_CAP, NUM_CAP],
                                  [1, DIM_CAP]]),
                            axis=AX.X, op=OP.add)
                    if it == 0:
                        nc.gpsimd.tensor_copy(bl_t[:], du[:])
                    else:
                        nc.gpsimd.tensor_tensor(bl_t[:], bl_t[:], du[:],
                                                op=OP.add)

            # final linear (one PSUM bank, three regions)
            pfin = ps_ef.tile([128, 3 * BL], F32, tag="pfin", bufs=1)
            nc.tensor.matmul(pfin[:, 0:BL], lhsT=out32[:, 0:128],
                             rhs=ident[:BL, :BL], start=True, stop=True)
            nc.tensor.matmul(pfin[:32, BL:2 * BL], lhsT=out32[:, 128:160],
                             rhs=ident[:BL, :BL], start=True, stop=True)
            capsT = rp.tile([128, 2 * BL], F32, tag="capsT")
            nc.vector.tensor_copy(capsT[:, 0:BL], pfin[:, 0:BL])
            nc.vector.tensor_copy(capsT[:32, BL:2 * BL], pfin[:32, BL:2 * BL])
            pf = pfin[0:2, 2 * BL:3 * BL]
            nc.tensor.matmul(pf, lhsT=wlin[:, 0, :], rhs=capsT[:, 0:BL],
                             start=True, stop=False)
            nc.tensor.matmul(pf, lhsT=wlin[:32, 1, :],
                             rhs=capsT[:32, BL:2 * BL],
                             start=False, stop=True)
            outT = rp.tile([2, BL], F32, tag="outT")
            nc.scalar.activation(outT[:], pf, AF.Identity,
                                 bias=blin[:, 0:1])
            dst = bass.AP(tensor=out_d, offset=0, ap=[[1, 2], [2, BL]])
            nc.sync.dma_start(out=dst, in_=outT[:])

    return nc


_CACHE = {}


def _get_nc(zero_bhn):
    if zero_bhn not in _CACHE:
        nc = _build(zero_bhn)
        _split_waits(nc)   # HW-path legalization (CoreSim path builds its own)
        _CACHE[zero_bhn] = nc
    return _CACHE[zero_bhn]


def _host_inputs(x, emb, w_ih_f, w_hh_f, b_ih_f, b_hh_f,
                 w_ih_b, w_hh_b, b_ih_b, b_hh_b, W_cap, W_lin, b_lin):
    """Build the per-core input maps (everything but xidx is shared)."""
    f32 = np.float32
    f16 = np.float16
    neg = np.ones((G3,), f32)
    neg[H:2 * H] = -1.0        # negate z gate (sigmoid -> 1-z)

    wih = np.stack([(w_ih_f.T * neg).astype(f16), (w_ih_b.T * neg).astype(f16)])
    whh = np.stack([(w_hh_f.T * neg).astype(f16), (w_hh_b.T * neg).astype(f16)])

    biasx = np.zeros((128, 6), f32)
    for d, (bi, bh) in enumerate([(b_ih_f, b_hh_f), (b_ih_b, b_hh_b)]):
        biasx[:, _BLKRZ[(d, 0)]] = (bi[0:H] + bh[0:H])
        biasx[:, _BLKRZ[(d, 1)]] = -(bi[H:2 * H] + bh[H:2 * H])
        biasx[:, 4 + d] = bi[2 * H:3 * H]
    bhn = np.zeros((128, 2), f32)
    bhn[:, 0] = b_hh_f[2 * H:3 * H]
    bhn[:, 1] = b_hh_b[2 * H:3 * H]
    zero_bhn = bool(np.all(bhn == 0.0))

    wcap = np.stack([W_cap[0:H, :].astype(f16), W_cap[H:2 * H, :].astype(f16)])
    selB = (np.arange(128)[:, None] % BL == np.arange(BL)[None, :]).astype(f16)
    selT = selB.T.copy().astype(f16)
    ident = np.eye(128, dtype=f32)

    shared = dict(emb=np.ascontiguousarray(emb).astype(f16), wih=wih, whh=whh,
                  biasx=biasx, bhn=bhn, wcap=wcap,
                  wlin=np.ascontiguousarray(W_lin, f32),
                  blin=np.ascontiguousarray(b_lin, f32).reshape(2, 1),
                  selB=selB, selT=selT, ident=ident)

    in_maps = []
    for c in range(NCORES):
        xl = np.asarray(x[c * BL:(c + 1) * BL, :])          # [BL, S]
        tok = xl.T.reshape(-1).astype(np.int32)             # s-major [NTOK]
        xidx = np.ascontiguousarray(tok.reshape(NGRP, 128).T)  # [128, NGRP]
        in_maps.append(dict(shared, xidx=xidx))
    return in_maps, zero_bhn


def kernel(**inputs):
    in_maps, zero_bhn = _host_inputs(**{k: np.asarray(v) for k, v in
                                        inputs.items()})
    nc = _get_nc(zero_bhn)
    res = run_bass_kernel_spmd(nc, in_maps, list(range(NCORES)))
    return np.concatenate([res.results[c]["out"] for c in range(NCORES)],
                          axis=0)


def _install_ntff_hook():
    """Shim the missing antenv.axon_hooks so trace=True works under axon."""
    import sys, types
    if "antenv.axon_hooks" in sys.modules:
        return
    mod = types.ModuleType("antenv.axon_hooks")
    _h = [None]
    mod.set_axon_ntff_profile_hook = lambda h: _h.__setitem__(0, h)
    mod.get_axon_ntff_profile_hook = lambda: _h[0]
    sys.modules["antenv.axon_hooks"] = mod
    import antenv
    antenv.axon_hooks = mod
    from trn_agent_boot.trn_boot import _ntff_profile_via_ctypes
    mod.set_axon_ntff_profile_hook(
        _ntff_profile_via_ctypes("/opt/axon/libaxon_pjrt.so"))


def kernel_profiled(**inputs):
    """Same as kernel() but with NTFF tracing; returns (out, result_obj)."""
    _install_ntff_hook()
    in_maps, zero_bhn = _host_inputs(**{k: np.asarray(v) for k, v in
                                        inputs.items()})
    nc = _get_nc(zero_bhn)
    res = run_bass_kernel_spmd(nc, in_maps, list(range(NCORES)), trace=True)
    out = np.concatenate([res.results[c]["out"] for c in range(NCORES)],
                         axis=0)
    return out, res


# revision 39
# speedup vs baseline: 1.2780x; 1.0716x over previous
"""Trainium2 Bass kernel for nn_CapRNNModelHelper (bi-GRU + capsule routing).

Sharding: data-parallel over batch across 8 cores (16 batch rows per core).
Everything else (embedding table, GRU weights, capsule weights) replicated.

Per-core pipeline (fp16 operands, f32 accumulation):
  1. indirect-DMA gather of fp16 embedding rows (token order s-major)
  2. PE-transpose (matmul vs identity) -> e.T  [300, ntok] fp16
  3. x_proj matmuls -> xprz + xpn (both fp16), biases folded, z negated
     so sigmoid gives w = 1-z directly
  4. chunk-parallel scan: PCH chunks per direction advance together in
     wide instructions; WU warmup steps rebuild each chunk's entry state
     from the previous chunk's tail (GRU forgets, so truncation error is
     tiny).  State h is fp16, written straight into the h buffer.
  5. capsule matmul (fp16) -> u_hat [sb, 160] fp16
  6. 5-iter dynamic routing; iteration 0 skips the c-multiply entirely
     (softmax is uniform and squash is scale-invariant)
  7. final linear -> out [16, 2]
"""

import numpy as np
from contextlib import ExitStack

import concourse.bass as bass
import concourse.tile as tile
from concourse import mybir
from concourse.bass import IndirectOffsetOnAxis
from concourse.bass_utils import run_bass_kernel_spmd
from concourse.tile_rust import add_dep_helper

F32 = mybir.dt.float32
F16 = mybir.dt.float16
F8 = mybir.dt.float8e4
I32 = mybir.dt.int32
AF = mybir.ActivationFunctionType
OP = mybir.AluOpType
AX = mybir.AxisListType

VOCAB, D_W, H, S, B = 50000, 300, 128, 256, 128
NUM_CAP, DIM_CAP, ROUTINGS, EPS = 10, 16, 5, 1e-7
NCORES = 8
BL = B // NCORES          # 16 batch rows per core
NTOK = S * BL             # 4096 tokens per core
NGRP = NTOK // 128        # 32 gather groups of 128 tokens
NCH = NTOK // 512         # 8 x_proj chunks of 512 tokens
KCH = [(0, 128), (128, 128), (256, 44)]   # D_W split
G3 = 3 * H                # 384

RZW = 4 * BL              # 64   per-step rz width [rf zf rb zb]
NW = 2 * BL               # 32   per-step n width [nf nb]
PCH = 16                  # parallel chunks per direction in the scan
CCH = S // PCH            # 16 steps per chunk
WU = 8                    # warmup steps (state rebuild) per chunk
EXT = S + 2 * WU          # padded xp timeline
PB = PCH * BL             # 256  scan state width per direction
NST = WU + CCH            # scan steps
# block index for (dir d, gate g): rz blocks 0..3, n blocks 0..1
_BLKRZ = {(0, 0): 0, (0, 1): 1, (1, 0): 2, (1, 1): 3}


def _sub(base, off, dims):
    """Manual AP: base is a [128, X] AP; append free dims after partition."""
    return bass.AP(tensor=base.tensor, offset=base.offset + off,
                   ap=[base.ap[0]] + dims)


def _v(t, dims, off=0):
    return bass.AP(tensor=t.tensor, offset=t.offset + off,
                   ap=[t.ap[0]] + dims)


def _split_waits(nc, cap=1):
    """Hoist excess sync waits onto standalone event-semaphore ops.

    The walrus build on this stack accepts only `cap` sync-wait commands
    per ISA instruction; Tile can attach several. Event-semaphore ops on
    the same engine execute in queue order, so hoisting preserves
    semantics.
    """
    n = 0
    for fn in nc.m.functions:
        for bb in fn.blocks:
            out = []
            for ins in bb.instructions:
                si = ins.sync_info
                if si is not None and len(si.on_wait) > cap:
                    waits = list(si.on_wait)
                    keep = waits[len(waits) - cap:] if cap else []
                    for w in waits[:len(waits) - cap] if cap else waits:
                        n += 1
                        out.append(mybir.InstEventSemaphore(
                            name=f"wsplit-{n}", engine=ins.engine,
                            ins=[], outs=[],
                            sync_info=mybir.SyncInfo(on_wait=[w],
                                                     on_update=[])))
                    ins.sync_info = mybir.SyncInfo(
                        on_wait=keep, on_update=list(si.on_update))
                out.append(ins)
            bb.instructions = out
    return n


def _build(zero_bhn: bool, debug: bool = False):
    nc = bass.Bass()
    xidx_d = nc.declare_dram_parameter("xidx", [128, NGRP], I32, False)
    emb_d = nc.declare_dram_parameter("emb", [VOCAB, D_W], F16, False)
    wih_d = nc.declare_dram_parameter("wih", [2, D_W, G3], F16, False)
    whh_d = nc.declare_dram_parameter("whh", [2, H, G3], F16, False)
    biasx_d = nc.declare_dram_parameter("biasx", [128, 6], F32, False)
    bhn_d = nc.declare_dram_parameter("bhn", [128, 2], F32, False)
    wcap_d = nc.declare_dram_parameter("wcap", [2, H, 160], F16, False)
    wlin_d = nc.declare_dram_parameter("wlin", [160, 2], F32, False)
    blin_d = nc.declare_dram_parameter("blin", [2, 1], F32, False)
    selB_d = nc.declare_dram_parameter("selB", [128, BL], F16, False)
    selT_d = nc.declare_dram_parameter("selT", [BL, 128], F16, False)
    ident_d = nc.declare_dram_parameter("ident", [128, 128], F32, False)
    out_d = nc.declare_dram_parameter("out", [BL, 2], F32, True)

    with tile.TileContext(nc) as tc, ExitStack() as ctx:
        const = ctx.enter_context(tc.tile_pool(name="const", bufs=1))
        bigxp = ctx.enter_context(tc.tile_pool(name="bigxp", bufs=1))
        bighs = ctx.enter_context(tc.tile_pool(name="bighs", bufs=1))
        work = ctx.enter_context(tc.tile_pool(name="work", bufs=3))

        # ---- constants to SBUF ----
        xidx = const.tile([128, NGRP], I32)
        nc.sync.dma_start(out=xidx[:], in_=xidx_d[:, :])
        whh = const.tile([128, 2, G3], F16)
        for d in range(2):
            nc.sync.dma_start(out=whh[:, d, :], in_=whh_d[d, :, :])
        biasx = const.tile([128, 6], F32)
        nc.sync.dma_start(out=biasx[:], in_=biasx_d[:, :])
        bhn = const.tile([128, 2], F32)
        nc.sync.dma_start(out=bhn[:], in_=bhn_d[:, :])
        wcap = const.tile([128, 2, 160], F16)
        for k in range(2):
            nc.sync.dma_start(out=wcap[:, k, :], in_=wcap_d[k, :, :])
        wlin = const.tile([128, 2, 2], F32)        # chunk0 [:128], chunk1 [:32]
        nc.sync.dma_start(out=wlin[:, 0, :], in_=wlin_d[0:128, :])
        nc.sync.dma_start(out=wlin[:32, 1, :], in_=wlin_d[128:160, :])
        blin = const.tile([2, 1], F32)
        nc.sync.dma_start(out=blin[:], in_=blin_d[:, :])
        selB = const.tile([128, BL], F16)
        nc.sync.dma_start(out=selB[:], in_=selB_d[:, :])
        selT = const.tile([BL, 128], F16)
        nc.sync.dma_start(out=selT[:], in_=selT_d[:, :])
        ident = const.tile([128, 128], F32)
        nc.sync.dma_start(out=ident[:], in_=ident_d[:, :])
        identb = const.tile([128, 128], F16)
        nc.scalar.copy(identb[:], ident[:])
        ident8 = const.tile([128, 128], F8)
        nc.scalar.copy(ident8[:], ident[:])
        epst = const.tile([128, 1], F32)
        nc.vector.memset(epst[:], EPS)

        xprz = bigxp.tile([128, EXT * RZW], F16)   # 35 KB/part
        xpn = bigxp.tile([128, EXT * NW], F16)     # 17.5 KB/part
        hbf = bighs.tile([128, 2 * S * BL], F16)   # 16 KB/part; f then b
        HB0 = S * BL
        # warmup pads force h -> 0 exactly: r=sigmoid(-30)=0, w=sigmoid(30)=1,
        # xn=0  =>  h' = h + w*(n - h) = n = tanh(0 + r*hn) = 0
        for p0 in (0, S + WU):
            for blk, val in ((0, -30.0), (1, 30.0), (2, -30.0), (3, 30.0)):
                nc.vector.memset(_sub(xprz[:], p0 * RZW + blk * BL,
                                      [[RZW, WU], [1, BL]]), val)
            nc.gpsimd.memset(_sub(xpn[:], p0 * NW, [[1, WU * NW]]), 0.0)

        # ---- phases B+C: gather + transpose + x_proj, 2 half passes ----
        HTOK = NTOK // 2
        with tc.tile_pool(name="bc", bufs=1) as bc, \
             tc.tile_pool(name="gat", bufs=16) as gat, \
             tc.tile_pool(name="ps_bc", bufs=1, space="PSUM") as ps_bc:
            wih = bc.tile([128, 2, 3, G3], F16)   # [kpart, dir, kchunk, gcol]
            for d in range(2):
                for k, (k0, kn) in enumerate(KCH):
                    nc.sync.dma_start(out=wih[:kn, d, k, :],
                                      in_=wih_d[d, k0:k0 + kn, :])
            for half in range(2):
                eT = [bc.tile([128, HTOK], F16, name=f"eT{k}", tag=f"eT{k}")
                      for k in range(3)]
                for i in range(NGRP // 2):
                    ig = half * (NGRP // 2) + i
                    g = gat.tile([128, D_W], F16, name="g", tag="g")
                    nc.gpsimd.indirect_dma_start(
                        out=g[:], out_offset=None,
                        in_=emb_d[:, :],
                        in_offset=IndirectOffsetOnAxis(ap=xidx[:, ig:ig + 1],
                                                       axis=0))
                    for k, (k0, kn) in enumerate(KCH):
                        pt = ps_bc.tile([128, 128], F32, tag="ptr", bufs=4)
                        nc.tensor.matmul(pt[:kn, :], lhsT=g[:, k0:k0 + kn],
                                         rhs=identb[:], start=True, stop=True)
                        if (i + k) % 2 == 0:
                            nc.vector.tensor_copy(
                                eT[k][:kn, i * 128:(i + 1) * 128], pt[:kn, :])
                        else:
                            nc.scalar.copy(
                                eT[k][:kn, i * 128:(i + 1) * 128], pt[:kn, :])
                for d in range(2):
                    for gt in range(3):
                        for ch in range(NCH // 2):
                            px = ps_bc.tile([128, 512], F32, tag="px", bufs=3)
                            for k, (k0, kn) in enumerate(KCH):
                                nc.tensor.matmul(
                                    px[:, :],
                                    lhsT=wih[:kn, d, k, gt * H:(gt + 1) * H],
                                    rhs=eT[k][:kn, ch * 512:(ch + 1) * 512],
                                    start=(k == 0), stop=(k == 2))
                            gch = half * (NCH // 2) + ch
                            src = _v(px, [[BL, 32], [1, BL]])
                            if gt < 2:
                                blk = _BLKRZ[(d, gt)]
                                dst = _sub(xprz[:], (WU + gch * 32) * RZW + blk * BL,
                                           [[RZW, 32], [1, BL]])
                                bcol = blk
                            else:
                                dst = _sub(xpn[:], (WU + gch * 32) * NW + d * BL,
                                           [[NW, 32], [1, BL]])
                                bcol = 4 + d
                            if (d * 3 + gt + ch) % 2 == 0:
                                nc.vector.tensor_scalar_add(
                                    dst, src, biasx[:, bcol:bcol + 1])
                            else:
                                nc.scalar.activation(
                                    dst, src, AF.Identity,
                                    bias=biasx[:, bcol:bcol + 1])

        # ---- phase D: chunked-parallel scan ----
        # PCH chunks per direction advance together in single wide
        # instructions; WU warmup slots rebuild each chunk's entry state from
        # the previous chunk's tail.  State is fp16 and written directly
        # into hbf once past warmup; during warmup it ping-pongs in hsc.
        hz = bighs.tile([128, 2 * PB], F16)
        nc.gpsimd.memset(hz[:], 0.0)
        hsc = [bighs.tile([128, 2 * PB], F16, name=f"hsc{i}") for i in range(2)]

        def h_off(d, k):
            """hbf column offset of direction d's state after step k>=WU."""
            j = k - WU
            return j * BL if d == 0 else HB0 + (CCH - 1 - j) * BL

        def h_fused(k):
            """Both directions' state after step k as one 3-dim AP."""
            if k < 0:
                return hz[:]
            if k < WU:
                return hsc[k % 2][:]
            o0 = h_off(0, k)
            return _sub(hbf[:], o0, [[h_off(1, k) - o0, 2],
                                     [CCH * BL, PCH], [1, BL]])

        def h_dir(d, k):
            """Direction d's state after step k (matmul rhs)."""
            if k < 0:
                return _sub(hz[:], d * PB, [[1, PB]])
            if k < WU:
                return _sub(hsc[k % 2][:], d * PB, [[1, PB]])
            return _sub(hbf[:], h_off(d, k), [[CCH * BL, PCH], [1, BL]])

        with tc.tile_pool(name="ps_scan", bufs=1, space="PSUM") as ps_sc:
            prev_b = prev_h = None
            for k in range(NST):
                for d in range(2):
                    prz = ps_sc.tile([128, 2 * PB], F32, tag=f"prz{d}",
                                     bufs=2)
                    pn = ps_sc.tile([128, PB], F32, tag=f"pn{d}", bufs=2)
                    xo = k if d == 0 else (CCH - 1 + 2 * WU - k)
                    xr_ap = _sub(xprz[:],
                                 xo * RZW + (2 * BL if d == 1 else 0),
                                 [[BL, 2], [CCH * RZW, PCH], [1, BL]])
                    hprev = h_dir(d, k - 1)
                    mi = nc.tensor.matmul(prz[:], lhsT=identb[:], rhs=xr_ap,
                                          start=True, stop=False)
                    g_r = nc.tensor.matmul(prz[:, 0:PB],
                                           lhsT=whh[:, d, 0:H], rhs=hprev,
                                           start=False, stop=False)
                    add_dep_helper(g_r.ins, mi.ins, sync=False, reason="acc")
                    g_z = nc.tensor.matmul(prz[:, PB:2 * PB],
                                           lhsT=whh[:, d, H:2 * H], rhs=hprev,
                                           start=False, stop=True)
                    add_dep_helper(g_z.ins, g_r.ins, sync=False, reason="acc")
                    nc.tensor.matmul(pn[:], lhsT=whh[:, d, 2 * H:3 * H],
                                     rhs=hprev, start=True, stop=True)

                    rw = work.tile([128, 2 * PB], F16, tag=f"rw{d}")
                    nc.scalar.activation(rw[:], prz[:], AF.Sigmoid)
                    # off-critical-path: a = w*h, c = h - a  (gpsimd)
                    a_t = work.tile([128, PB], F16, tag=f"a{d}")
                    nc.gpsimd.tensor_tensor(a_t[:], rw[:, PB:2 * PB], hprev,
                                            op=OP.mult)
                    c_t = work.tile([128, PB], F16, tag=f"c{d}")
                    nc.gpsimd.tensor_tensor(c_t[:], hprev, a_t[:],
                                            op=OP.subtract)
                    # critical path: tn = pn*r, t2 = tn+xn, n = tanh(t2),
                    # b = w*n, h' = c + b
                    tn = work.tile([128, PB], F16, tag=f"tn{d}")
                    if zero_bhn:
                        tni = nc.vector.tensor_tensor(tn[:], pn[:],
                                                      rw[:, 0:PB], op=OP.mult)
                    else:
                        tni = nc.vector.scalar_tensor_tensor(
                            tn[:], pn[:], bhn[:, d:d + 1], rw[:, 0:PB],
                            op0=OP.add, op1=OP.mult)
                    t2 = work.tile([128, PB], F16, tag=f"t2{d}")
                    xn_ap = _sub(xpn[:], xo * NW + d * BL,
                                 [[CCH * NW, PCH], [1, BL]])
                    nc.vector.tensor_add(_v(t2, [[BL, PCH], [1, BL]]),
                                         _v(tn, [[BL, PCH], [1, BL]]),
                                         xn_ap)
                    n_t = work.tile([128, PB], F16, tag=f"n{d}")
                    nc.scalar.activation(n_t[:], t2[:], AF.Tanh)
                    b_t = work.tile([128, PB], F16, tag=f"b{d}")
                    prev_b = nc.vector.tensor_tensor(b_t[:], rw[:, PB:2 * PB],
                                                     n_t[:], op=OP.mult)
                    heng = nc.vector if d == 0 else nc.gpsimd
                    prev_h = heng.tensor_tensor(h_dir(d, k), c_t[:], b_t[:],
                                                op=OP.add)

        # ---- phases E/F/G: capsule + routing + linear ----
        with tc.tile_pool(name="ef", bufs=1) as ef, \
             tc.tile_pool(name="rp", bufs=1) as rp, \
             tc.tile_pool(name="ps_ef", bufs=1, space="PSUM") as ps_ef:
            # capsule u_hat [sb, 160] fp16, and iteration-0 sequence sum
            # (uniform softmax + scale-invariant squash => no c multiply)
            uh = ef.tile([128, NGRP * 160], F16)
            # 4 parallel selB-reduction chains, one PSUM bank each
            # (512-f32 bank stride keeps each chain's accumulation region
            # in its own bank)
            CH4 = [(0, 8), (8, 8), (16, 8), (24, 8)]
            pos = ps_ef.tile([BL, 4 * 512], F32, tag="pos", bufs=1)

            def chains(rhs_of):
                for q, (c0, cn) in enumerate(CH4):
                    for j in range(cn):
                        nc.tensor.matmul(pos[:, q * 512:q * 512 + 160],
                                         lhsT=selB[:], rhs=rhs_of(c0 + j),
                                         start=(j == 0), stop=(j == cn - 1))

            for c in range(NGRP):
                pu = ps_ef.tile([128, 160], F32, tag="pu", bufs=2)
                nc.tensor.matmul(pu[:], lhsT=hbf[:, c * 128:(c + 1) * 128],
                                 rhs=wcap[:, 0, :], start=True, stop=False)
                nc.tensor.matmul(pu[:], lhsT=_sub(hbf[:], HB0 + c * 128,
                                                  [[1, 128]]),
                                 rhs=wcap[:, 1, :], start=False, stop=True)
                if c % 2 == 0:
                    nc.vector.tensor_copy(uh[:, c * 160:(c + 1) * 160], pu[:])
                else:
                    nc.scalar.copy(uh[:, c * 160:(c + 1) * 160], pu[:])
            chains(lambda c: uh[:, c * 160:(c + 1) * 160])

            # routing state
            bl_t = rp.tile([128, NGRP * NUM_CAP], F32, tag="bl")
            outputs = rp.tile([BL, 160], F16, tag="outs")
            out32 = rp.tile([BL, 160], F32, tag="out32")
            tmp = rp.tile([128, NGRP * 160], F16, tag="tmp")
            po = rp.tile([BL, 160], F32, tag="po", bufs=2)

            for it in range(ROUTINGS):
                if it > 0:
                    # softmax over capsules (no max-sub: |bl| is small)
                    sb_t = rp.tile([128, NGRP * NUM_CAP], F32, tag="sb",
                                   bufs=2)
                    nc.scalar.activation(sb_t[:], bl_t[:], AF.Exp)
                    sm = rp.tile([128, NGRP], F32, tag="sm", bufs=2)
                    nc.vector.tensor_reduce(
                        sm[:], _v(sb_t, [[NUM_CAP, NGRP], [1, NUM_CAP]]),
                        axis=AX.X, op=OP.add)
                    rc = rp.tile([128, NGRP], F32, tag="rc", bufs=2)
                    nc.vector.reciprocal(rc[:], sm[:])
                    c_t = rp.tile([128, NGRP * NUM_CAP], F16, tag="c",
                                  bufs=2)
                    nc.vector.tensor_tensor(
                        _v(c_t, [[NUM_CAP, NGRP], [1, NUM_CAP]]),
                        _v(sb_t, [[NUM_CAP, NGRP], [1, NUM_CAP]]),
                        _v(rc, [[1, NGRP], [0, NUM_CAP]]), op=OP.mult)
                    # tmp = u_hat * c (c broadcast over dc), then reduce
                    # over s: within-group via 4 parallel matmul chains,
                    # across partitions via selB.  Slice q's chain starts
                    # as soon as its multiply lands; gpsimd takes the last
                    # slice concurrently with vector's three.
                    for q, (c0, cn) in enumerate(CH4):
                        eng = nc.gpsimd if q == 3 else nc.vector
                        eng.tensor_tensor(
                            _sub(tmp[:], c0 * 160,
                                 [[160, cn], [DIM_CAP, NUM_CAP],
                                  [1, DIM_CAP]]),
                            _sub(uh[:], c0 * 160,
                                 [[160, cn], [DIM_CAP, NUM_CAP],
                                  [1, DIM_CAP]]),
                            _sub(c_t[:], c0 * NUM_CAP,
                                 [[NUM_CAP, cn], [1, NUM_CAP], [0, DIM_CAP]]),
                            op=OP.mult)
                        for j in range(cn):
                            nc.tensor.matmul(
                                pos[:, q * 512:q * 512 + 160], lhsT=selB[:],
                                rhs=tmp[:, (c0 + j) * 160:(c0 + j + 1) * 160],
                                start=(j == 0), stop=(j == cn - 1))
                nc.vector.tensor_reduce(po[:], _v(pos, [[1, 160], [512, 4]]),
                                        axis=AX.X, op=OP.add)
                # squash via ln/exp (same ACT table as Exp -> no reloads):
                # 1/sqrt(s+eps) = exp(-0.5*ln(s+eps))
                sq = rp.tile([BL, 160], F32, tag="sq", bufs=2)
                nc.scalar.square(sq[:], po[:])
                ssum = rp.tile([BL, NUM_CAP], F32, tag="ssum", bufs=2)
                nc.vector.tensor_reduce(
                    ssum[:], _v(sq, [[DIM_CAP, NUM_CAP], [1, DIM_CAP]]),
                    axis=AX.X, op=OP.add)
                lns = rp.tile([BL, NUM_CAP], F32, tag="lns", bufs=2)
                nc.scalar.activation(lns[:], ssum[:], AF.Ln,
                                     bias=epst[:BL, 0:1])
                rs = rp.tile([BL, NUM_CAP], F32, tag="rs", bufs=2)
                nc.scalar.activation(rs[:], lns[:], AF.Exp, scale=-0.5)
                nc.vector.tensor_tensor(
                    _v(outputs, [[DIM_CAP, NUM_CAP], [1, DIM_CAP]]),
                    _v(po, [[DIM_CAP, NUM_CAP], [1, DIM_CAP]]),
                    _v(rs, [[1, NUM_CAP], [0, DIM_CAP]]), op=OP.mult)
                if it == ROUTINGS - 1:
                    nc.gpsimd.tensor_tensor(
                        _v(out32, [[DIM_CAP, NUM_CAP], [1, DIM_CAP]]),
                        _v(po, [[DIM_CAP, NUM_CAP], [1, DIM_CAP]]),
                        _v(rs, [[1, NUM_CAP], [0, DIM_CAP]]), op=OP.mult)

                if it < ROUTINGS - 1:
                    # broadcast outputs to all 128 partitions via selT matmul
                    pob = ps_ef.tile([128, 160], F32, tag="pob", bufs=1)
                    nc.tensor.matmul(pob[:], lhsT=selT[:], rhs=outputs[:],
                                     start=True, stop=True)
                    ob = rp.tile([128, 160], F16, tag="ob", bufs=2)
                    nc.scalar.copy(ob[:], pob[:])
                    # tmp = u_hat * ob (ob broadcast over chunks), then
                    # du = sum over dc; interleave halves so the first du
                    # reduce overlaps the second multiply
                    du = rp.tile([128, NGRP * NUM_CAP], F32, tag="du", bufs=2)
                    for g0, gn in ((0, 16), (16, 16)):
                        nc.vector.tensor_tensor(
                            _sub(tmp[:], g0 * 160, [[160, gn], [1, 160]]),
                            _sub(uh[:], g0 * 160, [[160, gn], [1, 160]]),
                            _v(ob, [[0, gn], [1, 160]]),
                            op=OP.mult)
                        nc.vector.tensor_reduce(
                            _sub(du[:], g0 * NUM_CAP,
                                 [[NUM_CAP, gn], [1, NUM_CAP]]),
                            _sub(tmp[:], g0 * 160,
                                 [[160, gn], [DIM_CAP, NUM_CAP],
                                  [1, DIM_CAP]]),
                            axis=AX.X, op=OP.add)
                    if it == 0:
                        nc.gpsimd.tensor_copy(bl_t[:], du[:])
                    else:
                        nc.gpsimd.tensor_tensor(bl_t[:], bl_t[:], du[:],
                                                op=OP.add)

            # final linear (one PSUM bank, three regions)
            pfin = ps_ef.tile([128, 3 * BL], F32, tag="pfin", bufs=1)
            nc.tensor.matmul(pfin[:, 0:BL], lhsT=out32[:, 0:128],
                             rhs=ident[:BL, :BL], start=True, stop=True)
            nc.tensor.matmul(pfin[:32, BL:2 * BL], lhsT=out32[:, 128:160],
                             rhs=ident[:BL, :BL], start=True, stop=True)
            capsT = rp.tile([128, 2 * BL], F32, tag="capsT")
            nc.vector.tensor_copy(capsT[:, 0:BL], pfin[:, 0:BL])
            nc.vector.tensor_copy(capsT[:32, BL:2 * BL], pfin[:32, BL:2 * BL])
            pf = pfin[0:2, 2 * BL:3 * BL]
            nc.tensor.matmul(pf, lhsT=wlin[:, 0, :], rhs=capsT[:, 0:BL],
                             start=True, stop=False)
            nc.tensor.matmul(pf, lhsT=wlin[:32, 1, :],
                             rhs=capsT[:32, BL:2 * BL],
                             start=False, stop=True)
            outT = rp.tile([2, BL], F32, tag="outT")
            nc.scalar.activation(outT[:], pf, AF.Identity,
                                 bias=blin[:, 0:1])
            dst = bass.AP(tensor=out_d, offset=0, ap=[[1, 2], [2, BL]])
            nc.sync.dma_start(out=dst, in_=outT[:])

    return nc


_CACHE = {}


def _get_nc(zero_bhn):
    if zero_bhn not in _CACHE:
        nc = _build(zero_bhn)
        _split_waits(nc)   # HW-path legalization (CoreSim path builds its own)
        _CACHE[zero_bhn] = nc
    return _CACHE[zero_bhn]


def _host_inputs(x, emb, w_ih_f, w_hh_f, b_ih_f, b_hh_f,
                 w_ih_b, w_hh_b, b_ih_b, b_hh_b, W_cap, W_lin, b_lin):
    """Build the per-core input maps (everything but xidx is shared)."""
    f32 = np.float32
    f16 = np.float16
    neg = np.ones((G3,), f32)
    neg[H:2 * H] = -1.0        # negate z gate (sigmoid -> 1-z)

    wih = np.stack([(w_ih_f.T * neg).astype(f16), (w_ih_b.T * neg).astype(f16)])
    whh = np.stack([(w_hh_f.T * neg).astype(f16), (w_hh_b.T * neg).astype(f16)])

    biasx = np.zeros((128, 6), f32)
    for d, (bi, bh) in enumerate([(b_ih_f, b_hh_f), (b_ih_b, b_hh_b)]):
        biasx[:, _BLKRZ[(d, 0)]] = (bi[0:H] + bh[0:H])
        biasx[:, _BLKRZ[(d, 1)]] = -(bi[H:2 * H] + bh[H:2 * H])
        biasx[:, 4 + d] = bi[2 * H:3 * H]
    bhn = np.zeros((128, 2), f32)
    bhn[:, 0] = b_hh_f[2 * H:3 * H]
    bhn[:, 1] = b_hh_b[2 * H:3 * H]
    zero_bhn = bool(np.all(bhn == 0.0))

    wcap = np.stack([W_cap[0:H, :].astype(f16), W_cap[H:2 * H, :].astype(f16)])
    selB = (np.arange(128)[:, None] % BL == np.arange(BL)[None, :]).astype(f16)
    selT = selB.T.copy().astype(f16)
    ident = np.eye(128, dtype=f32)

    shared = dict(emb=np.ascontiguousarray(emb).astype(f16), wih=wih, whh=whh,
                  biasx=biasx, bhn=bhn, wcap=wcap,
                  wlin=np.ascontiguousarray(W_lin, f32),
                  blin=np.ascontiguousarray(b_lin, f32).reshape(2, 1),
                  selB=selB, selT=selT, ident=ident)

    in_maps = []
    for c in range(NCORES):
        xl = np.asarray(x[c * BL:(c + 1) * BL, :])          # [BL, S]
        tok = xl.T.reshape(-1).astype(np.int32)             # s-major [NTOK]
        xidx = np.ascontiguousarray(tok.reshape(NGRP, 128).T)  # [128, NGRP]
        in_maps.append(dict(shared, xidx=xidx))
    return in_maps, zero_bhn


def kernel(**inputs):
    in_maps, zero_bhn = _host_inputs(**{k: np.asarray(v) for k, v in
                                        inputs.items()})
    nc = _get_nc(zero_bhn)
    res = run_bass_kernel_spmd(nc, in_maps, list(range(NCORES)))
    return np.concatenate([res.results[c]["out"] for c in range(NCORES)],
                          axis=0)


def _install_ntff_hook():
    """Shim the missing antenv.axon_hooks so trace=True works under axon."""
    import sys, types
    if "antenv.axon_hooks" in sys.modules:
        return
    mod = types.ModuleType("antenv.axon_hooks")
    _h = [None]
    mod.set_axon_ntff_profile_hook = lambda h: _h.__setitem__(0, h)
    mod.get_axon_ntff_profile_hook = lambda: _h[0]
    sys.modules["antenv.axon_hooks"] = mod
    import antenv
    antenv.axon_hooks = mod
    from trn_agent_boot.trn_boot import _ntff_profile_via_ctypes
    mod.set_axon_ntff_profile_hook(
        _ntff_profile_via_ctypes("/opt/axon/libaxon_pjrt.so"))


def kernel_profiled(**inputs):
    """Same as kernel() but with NTFF tracing; returns (out, result_obj)."""
    _install_ntff_hook()
    in_maps, zero_bhn = _host_inputs(**{k: np.asarray(v) for k, v in
                                        inputs.items()})
    nc = _get_nc(zero_bhn)
    res = run_bass_kernel_spmd(nc, in_maps, list(range(NCORES)), trace=True)
    out = np.concatenate([res.results[c]["out"] for c in range(NCORES)],
                         axis=0)
    return out, res
